# revision 17
# baseline (speedup 1.0000x reference)
"""Trainium2 Bass kernel for a 2-layer Realformer-style cross-attention
transformer (B=8, S=1024, D=512, H=8, DFF=2048), data-parallel over batch
across 8 NeuronCores (one batch element per core, no collectives).

Layout: activations feature-major [D, S]; attention scores come out of the PE
[sk, sq] so softmax needs no transpose; V-heads carry a ones column so softmax
denominators fall out of the AV matmul.

v2 optimizations over the first working version:
- Realformer layer-1 score recompute folded into the same matmuls by stacking
  layer-0 q/k heads in partitions 64:128 of per-head tiles (contraction 128
  instead of 2x64): saves 128 matmuls.
- Software-pipelined emission: attention (ACT-bound: 64 exps per half) is
  interleaved with the previous half's O-proj/LN/FFN (PE-bound) so neither
  engine starves; FFN1+gelu is kept contiguous so the ACT engine switches
  activation-function sets only twice per block.
- LN 1/std via exp(-0.5*ln(var+eps)) (Ln+Exp live in the same ACT func set as
  attention's Exp -> no table reloads); softmax reciprocal via the fast
  custom-DVE approximation; x^2 for LN variance on the (idle) Pool engine.
"""

import sys

sys.path.insert(0, "/opt/trn_rl_repo")

import numpy as np
import ml_dtypes

B, S, D, H, HD, DFF, L = 8, 1024, 512, 8, 64, 2048, 2
P = 128
DC = D // P            # 4 d-chunks
FC = DFF // P          # 16 f-chunks
ST = S // P            # 8 seq tiles
NSQ = 2                # sq halves of 512
SQW = S // NSQ         # 512
EPS = 1e-5
N_CORES = 8

BF16 = ml_dtypes.bfloat16

_CACHE = {}


def _build_nc(repeats=1, ablate=(), mmn=512):
    import concourse.bacc as bacc
    import concourse.tile as tile
    from concourse import mybir
    from concourse.masks import make_identity

    f32 = mybir.dt.float32
    bf16 = mybir.dt.bfloat16
    AF = mybir.ActivationFunctionType
    OP = mybir.AluOpType
    AF_Gelu = AF.Identity if "acthack" in ablate else AF.Gelu
    AF_Sqrt = AF.Square if "acthack" in ablate else AF.Sqrt

    nc = bacc.Bacc(None, target_bir_lowering=False)

    # ---- external params ----
    xT_d = nc.declare_dram_parameter("xT", [D, S], bf16, isOutput=False)
    kT_d = nc.declare_dram_parameter("kT", [D, S], bf16, isOutput=False)
    vT_d = nc.declare_dram_parameter("vT", [D, S], bf16, isOutput=False)
    wq_d, wk_d, wv_d, wo_d, wf1_d, wf2_d = [], [], [], [], [], []
    bq_d, bk_d, bo_d, bf1_d, bf2_d, bf1n_d = [], [], [], [], [], []
    g1_d, b1_d, g2_d, b2_d = [], [], [], []
    for i in range(L):
        wq_d.append(nc.declare_dram_parameter(f"wq{i}", [D, D], bf16, isOutput=False))
        wk_d.append(nc.declare_dram_parameter(f"wk{i}", [D, D], bf16, isOutput=False))
        wv_d.append(nc.declare_dram_parameter(f"wv{i}", [D, D], bf16, isOutput=False))
        wo_d.append(nc.declare_dram_parameter(f"wo{i}", [D, D], bf16, isOutput=False))
        wf1_d.append(nc.declare_dram_parameter(f"wf1_{i}", [D, DFF], bf16, isOutput=False))
        wf2_d.append(nc.declare_dram_parameter(f"wf2_{i}", [DFF, D], bf16, isOutput=False))
        bq_d.append(nc.declare_dram_parameter(f"bq{i}", [D], f32, isOutput=False))
        bk_d.append(nc.declare_dram_parameter(f"bk{i}", [D], f32, isOutput=False))
        bo_d.append(nc.declare_dram_parameter(f"bo{i}", [D], f32, isOutput=False))
        bf1_d.append(nc.declare_dram_parameter(f"bf1_{i}", [DFF], f32, isOutput=False))
        bf1n_d.append(nc.declare_dram_parameter(f"bf1n_{i}", [DFF], f32, isOutput=False))
        bf2_d.append(nc.declare_dram_parameter(f"bf2_{i}", [D], f32, isOutput=False))
        g1_d.append(nc.declare_dram_parameter(f"g1_{i}", [D], f32, isOutput=False))
        b1_d.append(nc.declare_dram_parameter(f"b1_{i}", [D], f32, isOutput=False))
        g2_d.append(nc.declare_dram_parameter(f"g2_{i}", [D], f32, isOutput=False))
        b2_d.append(nc.declare_dram_parameter(f"b2_{i}", [D], f32, isOutput=False))
    y_d = nc.declare_dram_parameter("y", [S, D], f32, isOutput=True)

    with tile.TileContext(nc) as tc:
        import contextlib

        ctx = contextlib.ExitStack()
        with ctx:
            const = ctx.enter_context(tc.tile_pool(name="const", bufs=1))
            wpool = ctx.enter_context(tc.tile_pool(name="wpool", bufs=1))
            qk = ctx.enter_context(tc.tile_pool(name="qk", bufs=1))
            vhp = ctx.enter_context(tc.tile_pool(name="vhp", bufs=2))
            expp = ctx.enter_context(tc.tile_pool(name="expp", bufs=9))
            outp = ctx.enter_context(tc.tile_pool(name="outp", bufs=1))
            htp = ctx.enter_context(tc.tile_pool(name="htp", bufs=16))
            stream = ctx.enter_context(tc.tile_pool(name="stream", bufs=1))
            xsqp = ctx.enter_context(tc.tile_pool(name="xsqp", bufs=2))
            rowp = ctx.enter_context(tc.tile_pool(name="rowp", bufs=3))
            bcp = ctx.enter_context(tc.tile_pool(name="bcp", bufs=2))
            tmpp = ctx.enter_context(tc.tile_pool(name="tmpp", bufs=2))
            pp = ctx.enter_context(tc.tile_pool(name="pp", bufs=2, space="PSUM"))
            scp = ctx.enter_context(tc.tile_pool(name="scp", bufs=2, space="PSUM"))
            avp = ctx.enter_context(tc.tile_pool(name="avp", bufs=2, space="PSUM"))
            lnp = ctx.enter_context(tc.tile_pool(name="lnp", bufs=2, space="PSUM"))

            # ---- constants (loaded once, outside the repeat loop) ----
            ident_bf = const.tile([P, P], bf16, tag="ident_bf")
            make_identity(nc, ident_bf)
            ones_bf = const.tile([P, 1], bf16, tag="ones_bf")
            nc.vector.memset(ones_bf, 1.0)
            eps_t = const.tile([1, 1], f32, tag="eps")
            nc.vector.memset(eps_t, EPS)

            def load_cols(dram, n):
                t = const.tile([P, n], f32, tag=f"cols{dram.name}", name=f"c{dram.name}")
                nc.sync.dma_start(out=t, in_=dram[:].rearrange("(c p) -> p c", p=P))
                return t

            bq_t = [load_cols(bq_d[i], DC) for i in range(L)]
            bk_t = [load_cols(bk_d[i], DC) for i in range(L)]
            bo_t = [load_cols(bo_d[i], DC) for i in range(L)]
            bf1_t = [load_cols(bf1_d[i], FC) for i in range(L)]
            bf1n_t = [load_cols(bf1n_d[i], FC) for i in range(L)]
            bf2_t = [load_cols(bf2_d[i], DC) for i in range(L)]
            g1_t = [load_cols(g1_d[i], DC) for i in range(L)]
            b1_t = [load_cols(b1_d[i], DC) for i in range(L)]
            g2_t = [load_cols(g2_d[i], DC) for i in range(L)]
            b2_t = [load_cols(b2_d[i], DC) for i in range(L)]

            def load_fm(dram, tag):
                ts = []
                for c in range(DC):
                    t = const.tile([P, S], bf16, tag=f"{tag}{c}", name=f"{tag}{c}")
                    nc.sync.dma_start(out=t, in_=dram[c * P : (c + 1) * P, :])
                    ts.append(t)
                return ts

            xin = load_fm(xT_d, "xin")
            kt_t = load_fm(kT_d, "kin")
            vt_t = load_fm(vT_d, "vin")

            def build_body():
                # --- per-iteration persistent tiles ---
                qs = [qk.tile([P, S], bf16, tag=f"qs{h}", name=f"qs{h}")
                      for h in range(H)]
                ks = [qk.tile([P, S], bf16, tag=f"ks{h}", name=f"ks{h}")
                      for h in range(H)]
                W = [None, None]       # per-layer weight tiles
                vh = [None, None]      # per-layer V-head tiles
                outt = [None, None]    # attn output (feature-major), [li][sqh][dc]
                xcur = [[[xin[dc][:, sqh * SQW : (sqh + 1) * SQW] for dc in range(DC)]
                         for sqh in range(NSQ)], [None, None]]
                xln_s = [[None, None], [None, None]]   # LN1 out per (li, sqh)
                ht_s = [[None, None], [None, None]]    # FFN1 out per (li, sqh)
                x2_s = [[None, None], [None, None]]    # FFN residual per (li, sqh)
                ex_s = {}                              # (li, sqh, h) -> ex tiles

                def load_w(dram, nchunk, ncols, tg):
                    t = wpool.tile([P, nchunk, ncols], bf16, tag=tg, name=tg)
                    nc.sync.dma_start(
                        out=t, in_=dram[:].rearrange("(c p) e -> p c e", p=P)
                    )
                    return t

                def emit_weights_qkv(li):
                    W[li] = {
                        "wq": load_w(wq_d[li], DC, D, "wq"),
                        "wk": load_w(wk_d[li], DC, D, "wk"),
                        "wv": load_w(wv_d[li], DC, D, "wv"),
                    }

                def emit_weights_rest(li):
                    W[li]["wo"] = load_w(wo_d[li], DC, D, "wo")
                    W[li]["wf1"] = load_w(wf1_d[li], DC, DFF, "wf1")
                    W[li]["wf2"] = load_w(wf2_d[li], FC, D, "wf2")

                # Stacked q/k head layout: head h occupies rows 0:64 of its
                # tile for (layer parity) per head_of below; the layer-1
                # weights are host-permuted (64-col halves swapped inside each
                # 128 block) so every PSUM->SBUF write keeps its partition
                # range (no cross-partition moves).
                def head_of(li, et, hf):
                    return 2 * et + hf if li == 0 else 2 * et + 1 - hf

                def l0_rows(h):
                    return slice(0, HD) if h % 2 == 0 else slice(HD, P)

                def emit_proj_q(li, sqh):
                    sqsl = slice(sqh * SQW, (sqh + 1) * SQW)
                    for et in range(DC):
                        ps = pp.tile([P, SQW], f32, tag="pp", name=f"qp{li}{sqh}{et}")
                        for dc in range(DC):
                            nc.tensor.matmul(
                                ps, lhsT=W[li]["wq"][:, dc, et * P : (et + 1) * P],
                                rhs=xcur[li][sqh][dc],
                                start=(dc == 0), stop=(dc == DC - 1),
                            )
                        for hf in range(2):
                            rows = slice(hf * HD, (hf + 1) * HD)
                            nc.vector.tensor_scalar(
                                qs[head_of(li, et, hf)][rows, sqsl],
                                ps[rows, :],
                                bq_t[li][rows, et : et + 1],
                                None, OP.add,
                            )

                def emit_proj_k(li, ets):
                    for et in ets:
                        for sqh in range(NSQ):
                            sqsl = slice(sqh * SQW, (sqh + 1) * SQW)
                            ps = pp.tile([P, SQW], f32, tag="pp",
                                         name=f"kp{li}{sqh}{et}")
                            for dc in range(DC):
                                nc.tensor.matmul(
                                    ps, lhsT=W[li]["wk"][:, dc, et * P : (et + 1) * P],
                                    rhs=kt_t[dc][:, sqsl],
                                    start=(dc == 0), stop=(dc == DC - 1),
                                )
                            for hf in range(2):
                                rows = slice(hf * HD, (hf + 1) * HD)
                                nc.vector.tensor_scalar(
                                    ks[head_of(li, et, hf)][rows, sqsl],
                                    ps[rows, :],
                                    bk_t[li][rows, et : et + 1],
                                    None, OP.add,
                                )

                def emit_proj_v(li, sts):
                    if vh[li] is None:
                        vh[li] = [None] * ST
                    for st in sts:
                        ps = pp.tile([P, D], f32, tag="pp", name=f"vp{li}{st}")
                        for dc in range(DC):
                            nc.tensor.matmul(
                                ps, lhsT=vt_t[dc][:, st * P : (st + 1) * P],
                                rhs=W[li]["wv"][:, dc, :],
                                start=(dc == 0), stop=(dc == DC - 1),
                            )
                        t = vhp.tile([P, H, HD + 1], bf16, tag=f"vh{st}",
                                     name=f"vh{li}{st}")
                        nc.gpsimd.memset(t[:, :, HD : HD + 1], 1.0)
                        nc.vector.tensor_copy(
                            out=t[:, :, 0:HD],
                            in_=ps[:].rearrange("p (h w) -> p h w", h=H),
                        )
                        vh[li][st] = t

                # ---- attention units ----
                # Each unit emits scores+exp for head h interleaved (per
                # k-tile) with the AV accumulation of head h-1, so ex tiles
                # die one head after creation (small expp ring) and the PE
                # always has score work while ACT chews on exps.
                def unit_sa(li, sqh, h):
                    def f():
                        sqsl = slice(sqh * SQW, (sqh + 1) * SQW)
                        rows = l0_rows(h) if li == 0 else slice(0, P)
                        exs = []
                        hp = h - 1  # AV head
                        exp_prev = ex_s.pop((li, sqh, hp), None)
                        av = None
                        if exp_prev is not None:
                            av = avp.tile([HD + 1, SQW], f32, tag="av",
                                          name=f"av{li}{sqh}{hp}")
                        for kt in range(ST):
                            sc = scp.tile([P, SQW], f32, tag="sc",
                                          name=f"sc{li}{sqh}{h}{kt}")
                            nc.tensor.matmul(
                                sc, lhsT=ks[h][rows, kt * P : (kt + 1) * P],
                                rhs=qs[h][rows, sqsl], start=True, stop=True,
                            )
                            ex = expp.tile([P, SQW], bf16, tag="ex",
                                           name=f"ex{li}{sqh}{h}{kt}")
                            nc.scalar.activation(ex, sc, AF.Exp)
                            exs.append(ex)
                            if av is not None:
                                nc.tensor.matmul(
                                    av, lhsT=vh[li][kt][:, hp, :],
                                    rhs=exp_prev[kt],
                                    start=(kt == 0), stop=(kt == ST - 1),
                                )
                        if av is not None:
                            finish_av(li, sqh, hp, av)
                        ex_s[(li, sqh, h)] = exs
                    return f

                def finish_av(li, sqh, h, av):
                    pt, pb = h // 2, (h % 2) * HD
                    rec = rowp.tile([1, SQW], f32, tag="rec", bufs=1,
                                    name=f"rec{li}{sqh}{h}")
                    nc.vector.reciprocal(rec, av[HD : HD + 1, :])
                    bc = bcp.tile([HD, SQW], f32, tag="bca",
                                  name=f"bc{li}{sqh}{h}")
                    nc.gpsimd.partition_broadcast(bc, rec)
                    nc.vector.tensor_mul(
                        outt[li][sqh][pt][pb : pb + HD, :], av[0:HD, :], bc
                    )

                def unit_av_last(li, sqh):
                    def f():
                        h = H - 1
                        exs = ex_s.pop((li, sqh, h))
                        av = avp.tile([HD + 1, SQW], f32, tag="av",
                                      name=f"av{li}{sqh}{h}")
                        for kt in range(ST):
                            nc.tensor.matmul(
                                av, lhsT=vh[li][kt][:, h, :], rhs=exs[kt],
                                start=(kt == 0), stop=(kt == ST - 1),
                            )
                        finish_av(li, sqh, h, av)
                    return f

                def attn_units(li, sqh):
                    outt[li] = outt[li] or [None, None]
                    outt[li][sqh] = [
                        outp.tile([P, SQW], bf16, tag=f"ot{dc}",
                                  name=f"ot{li}{sqh}{dc}")
                        for dc in range(DC)
                    ]
                    units = [unit_sa(li, sqh, h) for h in range(H)]
                    units.append(unit_av_last(li, sqh))
                    return units

                # ---- post-chain units (for block that produced outt) ----
                def unit_oproj(li, sqh):
                    def f():
                        xnew = [stream.tile([P, SQW], bf16, tag=f"xnew{dc}",
                                            name=f"xnew{li}{sqh}{dc}")
                                for dc in range(DC)]
                        for ft in range(DC):
                            ps = pp.tile([P, SQW], f32, tag="pp",
                                         name=f"op{li}{sqh}{ft}")
                            for ec in range(DC):
                                nc.tensor.matmul(
                                    ps, lhsT=W[li]["wo"][:, ec, ft * P : (ft + 1) * P],
                                    rhs=outt[li][sqh][ec],
                                    start=(ec == 0), stop=(ec == DC - 1),
                                )
                            nc.vector.scalar_tensor_tensor(
                                xnew[ft], ps, bo_t[li][:, ft : ft + 1],
                                xcur[li][sqh][ft], OP.add, OP.add,
                            )
                        xln_s[li][sqh] = ("xnew", xnew)
                    return f

                def emit_ln_half(x_in, g_t, b_t, out_tag, li, sqh, final=False,
                                 obufs=1):
                    """x_in: 4 half-tiles [P, SQW]. Returns output tiles."""
                    sum_ps = lnp.tile([1, SQW], f32, tag="ln",
                                      name=f"su{out_tag}{li}{sqh}")
                    sq_ps = lnp.tile([1, SQW], f32, tag="ln",
                                     name=f"sq{out_tag}{li}{sqh}")
                    xsq = []
                    for dc in range(DC):
                        t = xsqp.tile([P, SQW], bf16, tag="xsq",
                                      name=f"xsq{out_tag}{li}{sqh}{dc}")
                        if "dvexsq" in ablate:
                            nc.vector.tensor_mul(t, x_in[dc], x_in[dc])
                        else:
                            nc.gpsimd.tensor_mul(t, x_in[dc], x_in[dc])
                        xsq.append(t)
                    for dc in range(DC):
                        nc.tensor.matmul(sum_ps, lhsT=ones_bf, rhs=x_in[dc],
                                         start=(dc == 0), stop=(dc == DC - 1))
                    for dc in range(DC):
                        nc.tensor.matmul(sq_ps, lhsT=ones_bf, rhs=xsq[dc],
                                         start=(dc == 0), stop=(dc == DC - 1))
                    mu = rowp.tile([1, SQW], f32, tag="mu", bufs=1,
                                   name=f"mu{out_tag}{li}{sqh}")
                    nc.vector.tensor_scalar(mu, sum_ps, 1.0 / D, None, OP.mult)
                    msq = rowp.tile([1, SQW], f32, tag="rows",
                                    name=f"ms{out_tag}{li}{sqh}")
                    nc.vector.tensor_mul(msq, mu, mu)
                    var = rowp.tile([1, SQW], f32, tag="rows",
                                    name=f"va{out_tag}{li}{sqh}")
                    nc.vector.scalar_tensor_tensor(
                        var, sq_ps, 1.0 / D, msq, OP.mult, OP.subtract
                    )
                    # mu broadcast starts as soon as mu exists (off the
                    # std/recip critical path); normalize as (x-mu)*a.
                    mu_bc = bcp.tile([P, SQW], f32, tag="cbc", bufs=1,
                                     name=f"mb{out_tag}{li}{sqh}")
                    nc.gpsimd.partition_broadcast(mu_bc, mu)
                    std = rowp.tile([1, SQW], f32, tag="rows",
                                    name=f"sd{out_tag}{li}{sqh}")
                    nc.scalar.activation(std, var, AF_Sqrt, bias=eps_t[0:1, :])
                    a_row = rowp.tile([1, SQW], f32, tag="rows",
                                      name=f"ar{out_tag}{li}{sqh}")
                    nc.vector.reciprocal_approx_fast(out=a_row, in_=std)
                    a_bc = bcp.tile([P, SQW], f32, tag="abc", bufs=1,
                                    name=f"ab{out_tag}{li}{sqh}")
                    nc.gpsimd.partition_broadcast(a_bc, a_row)
                    outs = []
                    for dc in range(DC):
                        t1 = tmpp.tile([P, SQW], bf16, tag="tmp",
                                       name=f"t1{out_tag}{li}{sqh}{dc}")
                        nc.vector.tensor_sub(t1, x_in[dc], mu_bc)
                        t2 = tmpp.tile([P, SQW], bf16, tag="tmp",
                                       name=f"t2{out_tag}{li}{sqh}{dc}")
                        nc.vector.tensor_mul(t2, t1, a_bc)
                        if final:
                            o = tmpp.tile([P, SQW], bf16, tag="fin", bufs=4,
                                          name=f"fin{sqh}{dc}")
                        else:
                            o = stream.tile([P, SQW], bf16, tag=f"{out_tag}{dc}",
                                            name=f"{out_tag}{li}{sqh}{dc}",
                                            bufs=obufs)
                        nc.scalar.activation(
                            o, t2, AF.Identity,
                            bias=b_t[:, dc : dc + 1], scale=g_t[:, dc : dc + 1],
                        )
                        outs.append(o)
                    return outs

                def unit_ln1(li, sqh):
                    def f():
                        _, xnew = xln_s[li][sqh]
                        xln = emit_ln_half(xnew, g1_t[li], b1_t[li], "xln", li, sqh)
                        xln_s[li][sqh] = ("xln", xln)
                    return f

                def unit_ffn1(li, sqh):
                    def f():
                        _, xln = xln_s[li][sqh]
                        hts = []
                        for ft in range(FC):
                            ps = pp.tile([P, SQW], f32, tag="pp",
                                         name=f"f1{li}{sqh}{ft}")
                            for dc in range(DC):
                                nc.tensor.matmul(
                                    ps, lhsT=W[li]["wf1"][:, dc, ft * P : (ft + 1) * P],
                                    rhs=xln[dc],
                                    start=(dc == 0), stop=(dc == DC - 1),
                                )
                            t = htp.tile([P, SQW], bf16, tag="ht",
                                         name=f"ht{li}{sqh}{ft}")
                            if "tablegelu" in ablate:
                                nc.scalar.activation(
                                    t, ps, AF_Gelu, bias=bf1_t[li][:, ft : ft + 1]
                                )
                            else:
                                # 2*gelu(x) ~= x*(1+tanh(0.851x)); Tanh shares
                                # the ACT func set with attention's Exp. The
                                # 0.5 is folded into Wf2 on the host.
                                th = htp.tile([P, SQW], bf16, tag="th", bufs=2,
                                              name=f"th{li}{sqh}{ft}")
                                nc.scalar.activation(
                                    th, ps, AF.Tanh, scale=0.851,
                                    bias=bf1n_t[li][:, ft : ft + 1],
                                )
                                u = htp.tile([P, SQW], bf16, tag="gu", bufs=2,
                                             name=f"gu{li}{sqh}{ft}")
                                nc.vector.scalar_tensor_tensor(
                                    u, ps, bf1_t[li][:, ft : ft + 1], th,
                                    OP.add, OP.mult,
                                )
                                nc.vector.scalar_tensor_tensor(
                                    t, ps, bf1_t[li][:, ft : ft + 1], u,
                                    OP.add, OP.add,
                                )
                            hts.append(t)
                        ht_s[li][sqh] = hts
                    return f

                def unit_ffn2(li, sqh, dts):
                    def f():
                        _, xln = xln_s[li][sqh]
                        if x2_s[li][sqh] is None:
                            x2_s[li][sqh] = [None] * DC
                        for dt in dts:
                            ps = pp.tile([P, SQW], f32, tag="pp",
                                         name=f"f2{li}{sqh}{dt}")
                            for fc in range(FC):
                                nc.tensor.matmul(
                                    ps, lhsT=W[li]["wf2"][:, fc, dt * P : (dt + 1) * P],
                                    rhs=ht_s[li][sqh][fc],
                                    start=(fc == 0), stop=(fc == FC - 1),
                                )
                            t = stream.tile([P, SQW], bf16, tag=f"x2_{dt}",
                                            name=f"x2{li}{sqh}{dt}")
                            nc.vector.scalar_tensor_tensor(
                                t, ps, bf2_t[li][:, dt : dt + 1],
                                xln[dt], OP.add, OP.add,
                            )
                            x2_s[li][sqh][dt] = t
                    return f

                def unit_ln2(li, sqh):
                    def f():
                        x2 = x2_s[li][sqh]
                        if li < L - 1:
                            xc = emit_ln_half(x2, g2_t[li], b2_t[li], "xc", li, sqh,
                                              obufs=2)
                            xcur[li + 1][sqh] = xc
                        else:
                            fins = emit_ln_half(x2, g2_t[li], b2_t[li], "fo",
                                                li, sqh, final=True)
                            x2_s[li][sqh] = fins  # stash for output unit
                    return f

                def unit_out(sqh):
                    def f():
                        fins = x2_s[L - 1][sqh]
                        for dc in range(DC):
                            for ss in range(SQW // P):
                                st_g = sqh * (SQW // P) + ss
                                tp = scp.tile([P, P], bf16, tag="sc",
                                              name=f"tp{sqh}{dc}{ss}")
                                nc.tensor.transpose(
                                    tp, fins[dc][:, ss * P : (ss + 1) * P], ident_bf
                                )
                                ysb = tmpp.tile([P, P], f32, tag="ysb", bufs=2,
                                                name=f"ysb{sqh}{dc}{ss}")
                                nc.vector.tensor_copy(out=ysb, in_=tp)
                                nc.sync.dma_start(
                                    out=y_d[st_g * P : (st_g + 1) * P,
                                            dc * P : (dc + 1) * P],
                                    in_=ysb,
                                )
                    return f

                def unit_proj_q(li, sqh):
                    return lambda: emit_proj_q(li, sqh)

                # ---- schedule ----
                def merge(attn, post):
                    """attn = [SA0..SA7, AVlast]; insert one post unit after
                    each unit from SA1 on, extras at the end."""
                    post = list(post)
                    out = []
                    for i, u in enumerate(attn):
                        out.append(u)
                        if i >= 1 and post:
                            out.append(post.pop(0))
                    out.extend(post)
                    return out

                def run(units):
                    for u in units:
                        u()

                # prologue: L0 weights, k/v/q projections
                emit_weights_qkv(0)
                emit_weights_rest(0)
                emit_proj_k(0, range(DC))
                emit_proj_v(0, range(ST))
                emit_proj_q(0, 0)

                # B1: attn(L0,s0) || qh0(s1), L1 qkv weights, kh1, vh1
                post_b1 = [
                    unit_proj_q(0, 1),
                    lambda: emit_weights_qkv(1),
                    lambda: emit_proj_k(1, range(0, 2)),
                    lambda: emit_proj_k(1, range(2, 4)),
                    lambda: emit_proj_v(1, range(0, 4)),
                    lambda: emit_proj_v(1, range(4, 8)),
                ]
                run(merge(attn_units(0, 0), post_b1))

                # B2: attn(L0,s1) || post(L0,s0)
                post_b2 = [
                    unit_oproj(0, 0), unit_ln1(0, 0), unit_ffn1(0, 0),
                    unit_ffn2(0, 0, (0, 1)), unit_ffn2(0, 0, (2, 3)),
                    unit_ln2(0, 0), unit_proj_q(1, 0),
                ]
                run(merge(attn_units(0, 1), post_b2))

                # B3: attn(L1,s0) || post(L0,s1) + L1 remaining weights
                post_b3 = [
                    unit_oproj(0, 1), unit_ln1(0, 1), unit_ffn1(0, 1),
                    unit_ffn2(0, 1, (0, 1)), unit_ffn2(0, 1, (2, 3)),
                    unit_ln2(0, 1), unit_proj_q(1, 1),
                    lambda: emit_weights_rest(1),
                ]
                run(merge(attn_units(1, 0), post_b3))

                # B4: attn(L1,s1) || post(L1,s0) + output(s0)
                post_b4 = [
                    unit_oproj(1, 0), unit_ln1(1, 0), unit_ffn1(1, 0),
                    unit_ffn2(1, 0, (0, 1)), unit_ffn2(1, 0, (2, 3)),
                    unit_ln2(1, 0), unit_out(0),
                ]
                run(merge(attn_units(1, 1), post_b4))

                # tail: post(L1,s1) + output(s1)
                run([
                    unit_oproj(1, 1), unit_ln1(1, 1), unit_ffn1(1, 1),
                    unit_ffn2(1, 1, (0, 1)), unit_ffn2(1, 1, (2, 3)),
                    unit_ln2(1, 1), unit_out(1),
                ])

            if repeats == 1:
                build_body()
            else:
                with tc.For_i(0, repeats, 1,
                              hint_engines=(mybir.EngineType.Pool,
                                            mybir.EngineType.Activation,
                                            mybir.EngineType.PE,
                                            mybir.EngineType.DVE,
                                            mybir.EngineType.SP)):
                    build_body()

    nc.compile()
    return nc


def _prep_inputs(inputs):
    """Host-side folding + sharding. Returns per-core in_maps."""
    f = {k: np.asarray(v, dtype=np.float32) for k, v in inputs.items()}
    q, k, v = f["q"], f["k"], f["v"]
    # layer-1 q/k head permutation: swap the two 64-col halves inside each
    # 128-col block so PSUM rows land on the right half of the stacked tiles
    hperm = np.arange(D).reshape(DC, 2, HD)[:, ::-1, :].reshape(-1)
    maps_common = {}
    for i in range(L):
        eff = f["scale"][i] * np.clip(f["extra_scale"][i], 0.01, 50.0)
        sp_a = np.log1p(np.exp(f["gate_attn"][i]))
        sp_f = np.log1p(np.exp(f["gate_ffn"][i]))
        WQi, WKi, bQi, bKi = f["WQ"][i], f["WK"][i], f["bQ"][i], f["bK"][i]
        if i == 1:
            WQi, WKi = WQi[:, hperm], WKi[:, hperm]
            bQi, bKi = bQi[hperm], bKi[hperm]
        wq = (WQi * eff).astype(BF16)
        wk = WKi.astype(BF16)
        wv = f["WV"][i].astype(BF16)
        wo = (f["WO"][i] * sp_a).astype(BF16)
        wf1 = f["Wf1"][i].astype(BF16)
        wf2 = (f["Wf2"][i] * sp_f * 0.5).astype(BF16)
        bq = (bQi * eff).astype(np.float32)
        bk = bKi.astype(np.float32)
        # fold V bias through O projection: (out + bV) @ WO + bO
        bo = (sp_a * (f["bO"][i] + f["bV"][i] @ f["WO"][i])).astype(np.float32)
        bf1 = f["bf1"][i].astype(np.float32)
        bf1n = (0.851 * f["bf1"][i]).astype(np.float32)
        bf2 = (f["bf2"][i] * sp_f).astype(np.float32)
        maps_common.update({
            f"wq{i}": wq, f"wk{i}": wk, f"wv{i}": wv, f"wo{i}": wo,
            f"wf1_{i}": wf1, f"wf2_{i}": wf2,
            f"bq{i}": bq, f"bk{i}": bk, f"bo{i}": bo,
            f"bf1_{i}": bf1, f"bf2_{i}": bf2, f"bf1n_{i}": bf1n,
            f"g1_{i}": f["ln1_g"][i].astype(np.float32),
            f"b1_{i}": f["ln1_b"][i].astype(np.float32),
            f"g2_{i}": f["ln2_g"][i].astype(np.float32),
            f"b2_{i}": f["ln2_b"][i].astype(np.float32),
        })
    in_maps = []
    for b in range(B):
        m = dict(maps_common)
        m["xT"] = np.ascontiguousarray(q[b].T).astype(BF16)
        m["kT"] = np.ascontiguousarray(k[b].T).astype(BF16)
        m["vT"] = np.ascontiguousarray(v[b].T).astype(BF16)
        in_maps.append(m)
    return in_maps


def get_nc(repeats=1, ablate=(), mmn=512):
    key = ("nc", repeats, tuple(ablate), mmn)
    if key not in _CACHE:
        _CACHE[key] = _build_nc(repeats, ablate=tuple(ablate), mmn=mmn)
    return _CACHE[key]


def kernel(**inputs) -> np.ndarray:
    from concourse.bass_utils import run_bass_kernel_spmd

    nc = get_nc()
    in_maps = _prep_inputs(inputs)
    res = run_bass_kernel_spmd(nc, in_maps, core_ids=list(range(N_CORES)))
    out = np.stack([res.results[b]["y"] for b in range(B)], axis=0)
    return out.astype(np.float32)
